# revision 10
# baseline (speedup 1.0000x reference)
"""Trainium2 Bass kernel for the B-spline (KAN-style) layer.

Computes out[b,f] = sum_k basis_k(x[b,f]) * control_p[k,f] + bias[f] where
basis is the cubic B-spline basis from the reference (64 functions, knots
uniform on [0,1] with spacing 1/55 plus boundary extension knots).

Algorithm ("telescoped clamp"): in s = 55*x coordinates the spline S_f(s) is a
C^2 piecewise cubic with integer knots 1..54 inside the domain.  Writing the
piece on [j, j+1) as g0 + g1*u + g2*u^2 + g3*u^3 (u = s - j), continuity gives

    S(s) = S(0) + sum_{j'=0}^{54} D_{j'}(clamp(s - j', 0, 1)),
    D_{j'}(v) = a_{j'} v + b_{j'} v^2 + c_{j'} v^3   (no constant term),

which needs no data-dependent table lookup: 55 clamped-cubic terms, each
mapped to two fused 7-stage custom DVE instructions with per-partition
(= per-feature) coefficient scalars.  All terms and partial sums are O(1)
(partial sums equal S at the knots), so fp32 accumulation is stable.

Sharding: data-parallel over batch (4 slices) x features (2 halves) = 8 cores.
Each core handles a (1024 batch, 128 feature) shard; no collectives needed.
Per-feature coefficient tables are built on-device by one small matmul from
control_p against a fixed host constant.
"""

import sys

if "/opt/trn_rl_repo" not in sys.path:
    sys.path.insert(0, "/opt/trn_rl_repo")

import numpy as np

import concourse.bass as bass
import concourse.bacc as bacc
import concourse.tile as tile
from concourse import mybir
from concourse.bass_utils import run_bass_kernel_spmd

BATCH, NF, NK, DG = 4096, 256, 64, 3
NJ = 55          # spline intervals covering x in [0,1)
NCORES = 8
BSH, FSH = 1024, 128   # per-core shard: batch x features
F32 = mybir.dt.float32

# ---------------------------------------------------------------------------
# Host-side spline tables (float64, exact)
# ---------------------------------------------------------------------------

def _knots64():
    dg, nk = DG, NK
    base = np.concatenate([
        np.linspace(-0.002, -0.001, dg),
        np.linspace(0.0, 1.0, nk - 2 * dg - 2),
        np.linspace(1.001, 1.002, dg),
    ])
    dist_lo = base[1] - base[0]
    dist_hi = base[-1] - base[-2]
    left = base[0] - dist_lo * np.arange(dg, 0, -1)
    right = base[-1] + dist_hi * np.arange(1, dg + 1)
    t32 = np.concatenate([left, base, right]).astype(np.float32)
    return t32.astype(np.float64)


def _basis64(x, t):
    xe = x[..., None]
    B = ((t[:-1] <= xe) & (xe < t[1:])).astype(np.float64)
    for k in range(1, DG + 1):
        d1 = t[k:-1] - t[:-k - 1]
        d2 = t[k + 1:] - t[1:-k]
        w1 = np.where(d1 != 0, (xe - t[:-k - 1]) / np.where(d1 != 0, d1, 1.0), 0.0)
        w2 = np.where(d2 != 0, (t[k + 1:] - xe) / np.where(d2 != 0, d2, 1.0), 0.0)
        B = w1 * B[..., :-1] + w2 * B[..., 1:]
    return B  # (..., 64)


def _make_w2():
    """Constant (65, 166) matrix W2 such that  cpb.T @ W2  gives per-feature
    chain coefficients, where cpb = [control_p_shard; bias_shard] is (65, F).

    Columns: j' in 0..54 -> a_{j'};  55+j' -> b_{j'};  110+j' -> c_{j'};
    165 -> S(0) + bias  (accumulator init).
    """
    t = _knots64()
    us = np.array([0.15, 0.35, 0.65, 0.85])
    V = np.vander(us, 4, increasing=True)
    Vinv = np.linalg.inv(V)
    w2 = np.zeros((NK + 1, 3 * NJ + 1), dtype=np.float64)
    for j in range(NJ):
        xs = (j + us) / 55.0
        Bs = _basis64(xs, t)           # (4, 64)
        for ii in range(4):            # active basis funcs j+3 .. j+6
            coef = Vinv @ Bs[:, j + 3 + ii]   # poly coeffs in u, degree 0..3
            # device computes v = 2*clamp(s - j', 0, 1); pre-scale by 2^-d
            w2[j + 3 + ii, j] += coef[1] / 2.0         # a'
            w2[j + 3 + ii, 55 + j] += coef[2] / 4.0    # b'
            w2[j + 3 + ii, 110 + j] += coef[3] / 8.0   # c'
            if j == 0:
                w2[3 + ii, 165] += coef[0]         # S(0) constant
    w2[NK, 165] = 1.0                               # bias row
    return np.ascontiguousarray(w2, dtype=np.float32)


# ---------------------------------------------------------------------------
# Custom DVE ops:  v = min(relu(s - j'), 1)  then chained cubic accumulate
# ---------------------------------------------------------------------------

def _register_ops():
    """Two chained 8-stage ops.  The DVE carry-lane budget allows only 6
    distinct leaves, so the clamp is built without the Zero constant:
    v = min(t + |t|, 2) = 2*clamp(t, 0, 1); host pre-scales coefficients
    by 1/2^d to compensate."""
    from concourse import dve_ops
    from concourse.dve_spec import (
        Spec, Src0, Src1, C0, C1, C2, One, minn, sq, lower, Bin, AluOp,
        _has_src1 as has_src1,
    )
    from concourse.dve_uop import DveOpSpec

    if any(op.name == "BSPL_AB_ANT" for op in dve_ops.OPS):
        ab = next(op for op in dve_ops.OPS if op.name == "BSPL_AB_ANT")
        cc = next(op for op in dve_ops.OPS if op.name == "BSPL_C_ANT")
        return ab, cc

    t1 = Src0 - C2
    v1 = minn(t1 + Bin(AluOp.ABSOLUTE_VALUE, t1, t1), One + One)
    body_ab = Src1 + v1 * (C0 + C1 * v1)        # acc + a'*v + b'*v^2
    t2 = Src0 - C2
    v2 = minn(t2 + Bin(AluOp.ABSOLUTE_VALUE, t2, t2), One + One)
    body_c = Src1 + (sq(v2) * v2) * C0          # acc + c'*v^3

    def _vv(in0, imm2):
        tt = in0.astype(np.float32) - np.float32(imm2)
        return np.minimum(tt + np.abs(tt), np.float32(2.0)).astype(np.float32)

    def ref_ab(in0, in1, s0, s1, imm2):
        vv = _vv(in0, imm2)
        return (in1 + vv * (s0 + s1 * vv)).astype(np.float32)

    def ref_c(in0, in1, s0, s1, imm2):
        vv = _vv(in0, imm2)
        return (in1 + (vv * vv * vv) * s0).astype(np.float32)

    def _mk(name, spec):
        # compute the pinned table hashes for this repo's lowerer
        shas = {}
        for ver in ("v3", "v4"):
            probe = DveOpSpec(name=name, opcode=0,
                              uops=lower(spec, ver=ver), rd1_en=has_src1(spec))
            shas[ver] = probe.sha(ver)
        op = dve_ops.DveOp(name, spec, subdim=False, uops_sha=shas)
        dve_ops.OPS.append(op)
        dve_ops.CUSTOM_DVE_SPECS[name] = spec
        row = dve_ops._CUSTOM_DVE_ROW_BASE + len(dve_ops.OPS) - 1
        assert row < 0x20
        dve_ops._SUB_OPCODE_FOR_NAME[name] = row
        return op

    ab = _mk("BSPL_AB_ANT", Spec(body=body_ab, reference=ref_ab))
    cc = _mk("BSPL_C_ANT", Spec(body=body_c, reference=ref_c))
    return ab, cc


# ---------------------------------------------------------------------------
# Bass kernel
# ---------------------------------------------------------------------------

_CACHE = {}


def _build_module(body_reps=1):
    if "nc" in _CACHE:
        return _CACHE["nc"]
    op_ab, op_c = _register_ops()
    from concourse import masks

    nc = bacc.Bacc("TRN2", target_bir_lowering=False, debug=False,
                   num_devices=NCORES)
    x_in = nc.dram_tensor("x", [BSH, FSH], F32, kind="ExternalInput").ap()
    cpb_in = nc.dram_tensor("cpb", [NK + 1, FSH], F32, kind="ExternalInput").ap()
    w2_in = nc.dram_tensor("w2", [NK + 1, 3 * NJ + 1], F32,
                           kind="ExternalInput").ap()
    y_out = nc.dram_tensor("y", [BSH, FSH], F32, kind="ExternalOutput").ap()

    NCHUNK = BSH // 128   # 8 transpose chunks

    import contextlib
    with tile.TileContext(nc) as tc:
        with contextlib.ExitStack() as _st:
            const_pool = _st.enter_context(tc.tile_pool(name="const", bufs=1))
            xin_pool = _st.enter_context(tc.tile_pool(name="xin", bufs=4))
            big_pool = _st.enter_context(tc.tile_pool(name="big", bufs=1))
            psum_pool = _st.enter_context(
                tc.tile_pool(name="ps", bufs=2, space="PSUM"))
            psum_out_pool = _st.enter_context(
                tc.tile_pool(name="pso", bufs=2, space="PSUM"))
            psum_g_pool = _st.enter_context(
                tc.tile_pool(name="psg", bufs=1, space="PSUM"))
            if body_reps > 1:
                _st.enter_context(tc.For_i(0, body_reps, 1))
            # --- coefficient table: gtab[f, col] = (cpb.T @ w2)[f, col] ---
            cpb_sb = const_pool.tile([NK + 1, FSH], F32)
            nc.sync.dma_start(cpb_sb[:], cpb_in[:])
            w2_sb = const_pool.tile([NK + 1, 3 * NJ + 1], F32)
            nc.sync.dma_start(w2_sb[:], w2_in[:])
            g_ps = psum_g_pool.tile([FSH, 3 * NJ + 1], F32)
            nc.tensor.matmul(g_ps[:], cpb_sb[:], w2_sb[:])
            gtab = const_pool.tile([FSH, 3 * NJ + 1], F32)
            nc.scalar.copy(gtab[:], g_ps[:])

            ident = const_pool.tile([128, 128], F32)
            masks.make_identity(nc, ident[:])

            # --- load x, transpose to (feature, batch), scale to s = 55 x ---
            s_t = big_pool.tile([FSH, BSH], F32)
            for i in range(NCHUNK):
                xt = xin_pool.tile([128, FSH], F32)
                nc.sync.dma_start(xt[:], x_in[bass.ts(i, 128), :])
                pt = psum_pool.tile([FSH, 128], F32)
                nc.tensor.transpose(pt[:], xt[:], ident[:])
                nc.scalar.mul(s_t[:, bass.ts(i, 128)], pt[:], 55.0)

            # --- telescoped chains ---
            acc_ab = big_pool.tile([FSH, BSH], F32)
            acc_c = big_pool.tile([FSH, BSH], F32)
            nc.scalar.activation(acc_ab[:], s_t[:],
                                 mybir.ActivationFunctionType.Identity,
                                 bias=gtab[:, 165:166], scale=0.0)
            nc.gpsimd.memset(acc_c[:], 0.0)
            for jp in range(NJ):
                nc.vector._custom_dve(
                    op_ab, out=acc_ab[:], in0=s_t[:], in1=acc_ab[:],
                    s0=gtab[:, jp:jp + 1], s1=gtab[:, 55 + jp:56 + jp],
                    imm2=float(jp))
                nc.vector._custom_dve(
                    op_c, out=acc_c[:], in0=s_t[:], in1=acc_c[:],
                    s0=gtab[:, 110 + jp:111 + jp], imm2=float(jp))
            nc.vector.tensor_add(acc_ab[:], acc_ab[:], acc_c[:])

            # --- transpose back and store ---
            for i in range(NCHUNK):
                po = psum_out_pool.tile([128, FSH], F32)
                nc.tensor.transpose(po[:], acc_ab[:, bass.ts(i, 128)], ident[:])
                yo = xin_pool.tile([128, FSH], F32, tag="yout")
                nc.scalar.copy(yo[:], po[:])
                nc.sync.dma_start(y_out[bass.ts(i, 128), :], yo[:])

    nc.compile()
    _CACHE["nc"] = nc
    return nc


# ---------------------------------------------------------------------------
# Public entry point
# ---------------------------------------------------------------------------

def _make_in_maps(x, control_p, bias):
    x = np.ascontiguousarray(x, dtype=np.float32)
    control_p = np.ascontiguousarray(control_p, dtype=np.float32)
    bias = np.ascontiguousarray(bias, dtype=np.float32)
    assert x.shape == (BATCH, NF) and control_p.shape == (NK, NF)
    w2 = _make_w2()
    in_maps, slots = [], []
    for c in range(NCORES):
        fh, bq = c // 4, c % 4
        fsl = slice(fh * FSH, (fh + 1) * FSH)
        bsl = slice(bq * BSH, (bq + 1) * BSH)
        cpb = np.concatenate([control_p[:, fsl], bias[None, fsl]], axis=0)
        in_maps.append({
            "x": np.ascontiguousarray(x[bsl, fsl]),
            "cpb": np.ascontiguousarray(cpb),
            "w2": w2,
        })
        slots.append((bsl, fsl))
    return in_maps, slots


def kernel(x, control_p, bias):
    nc = _build_module()
    in_maps, slots = _make_in_maps(x, control_p, bias)
    res = run_bass_kernel_spmd(nc, in_maps, list(range(NCORES)))

    out = np.empty((BATCH, NF), dtype=np.float32)
    for c, (bsl, fsl) in enumerate(slots):
        out[bsl, fsl] = res.results[c]["y"]
    return out


# revision 21
# speedup vs baseline: 1.5563x; 1.5563x over previous
"""Trainium2 Bass kernel for the B-spline (KAN-style) layer.

Computes out[b,f] = sum_k basis_k(x[b,f]) * control_p[k,f] + bias[f] where
basis is the cubic B-spline basis from the reference (64 functions, knots
uniform on [0,1] with spacing 1/55 plus boundary extension knots).

Algorithm ("telescoped clamp"): in s = 55*x coordinates the spline S_f(s) is a
C^2 piecewise cubic with integer knots 1..54 inside the domain.  Writing the
piece on [j, j+1) as g0 + g1*u + g2*u^2 + g3*u^3 (u = s - j), continuity gives

    S(s) = S(0) + sum_{j'=0}^{54} D_{j'}(clamp(s - j', 0, 1)),
    D_{j'}(v) = a_{j'} v + b_{j'} v^2 + c_{j'} v^3   (no constant term),

which needs no data-dependent table lookup: 55 clamped-cubic terms, each
mapped to two fused 7-stage custom DVE instructions with per-partition
(= per-feature) coefficient scalars.  All terms and partial sums are O(1)
(partial sums equal S at the knots), so fp32 accumulation is stable.

Sharding: data-parallel over batch (4 slices) x features (2 halves) = 8 cores.
Each core handles a (1024 batch, 128 feature) shard; no collectives needed.
Per-feature coefficient tables are built on-device by one small matmul from
control_p against a fixed host constant.
"""

import sys

if "/opt/trn_rl_repo" not in sys.path:
    sys.path.insert(0, "/opt/trn_rl_repo")

import numpy as np

import concourse.bass as bass
import concourse.bacc as bacc
import concourse.tile as tile
from concourse import mybir
from concourse.bass_utils import run_bass_kernel_spmd

BATCH, NF, NK, DG = 4096, 256, 64, 3
NJ = 55          # spline intervals covering x in [0,1)
NCORES = 8
BSH, FSH = 1024, 128   # per-core shard: batch x features
F32 = mybir.dt.float32

# ---------------------------------------------------------------------------
# Host-side spline tables (float64, exact)
# ---------------------------------------------------------------------------

def _knots64():
    dg, nk = DG, NK
    base = np.concatenate([
        np.linspace(-0.002, -0.001, dg),
        np.linspace(0.0, 1.0, nk - 2 * dg - 2),
        np.linspace(1.001, 1.002, dg),
    ])
    dist_lo = base[1] - base[0]
    dist_hi = base[-1] - base[-2]
    left = base[0] - dist_lo * np.arange(dg, 0, -1)
    right = base[-1] + dist_hi * np.arange(1, dg + 1)
    t32 = np.concatenate([left, base, right]).astype(np.float32)
    return t32.astype(np.float64)


def _basis64(x, t):
    xe = x[..., None]
    B = ((t[:-1] <= xe) & (xe < t[1:])).astype(np.float64)
    for k in range(1, DG + 1):
        d1 = t[k:-1] - t[:-k - 1]
        d2 = t[k + 1:] - t[1:-k]
        w1 = np.where(d1 != 0, (xe - t[:-k - 1]) / np.where(d1 != 0, d1, 1.0), 0.0)
        w2 = np.where(d2 != 0, (t[k + 1:] - xe) / np.where(d2 != 0, d2, 1.0), 0.0)
        B = w1 * B[..., :-1] + w2 * B[..., 1:]
    return B  # (..., 64)


def _make_w2():
    """Constant (65, 166) matrix W2 such that  cpb.T @ W2  gives per-feature
    chain coefficients, where cpb = [control_p_shard; bias_shard] is (65, F).

    Columns: j' in 0..54 -> a_{j'};  55+j' -> b_{j'};  110+j' -> c_{j'};
    165 -> S(0) + bias  (accumulator init).
    """
    t = _knots64()
    us = np.array([0.15, 0.35, 0.65, 0.85])
    V = np.vander(us, 4, increasing=True)
    Vinv = np.linalg.inv(V)
    w2 = np.zeros((NK + 1, 3 * NJ + 1), dtype=np.float64)
    for j in range(NJ):
        xs = (j + us) / 55.0
        Bs = _basis64(xs, t)           # (4, 64)
        for ii in range(4):            # active basis funcs j+3 .. j+6
            coef = Vinv @ Bs[:, j + 3 + ii]   # poly coeffs in u, degree 0..3
            # device computes v = 2*clamp(s - j', 0, 1); pre-scale by 2^-d
            w2[j + 3 + ii, j] += coef[1] / 2.0         # a'
            w2[j + 3 + ii, 55 + j] += coef[2] / 4.0    # b'
            w2[j + 3 + ii, 110 + j] += coef[3] / 8.0   # c'
            if j == 0:
                w2[3 + ii, 165] += coef[0]         # S(0) constant
    w2[NK, 165] = 1.0                               # bias row
    return np.ascontiguousarray(w2, dtype=np.float32)


# ---------------------------------------------------------------------------
# Custom DVE ops:  v = min(relu(s - j'), 1)  then chained cubic accumulate
# ---------------------------------------------------------------------------

def _register_ops():
    """Two chained 8-stage ops.  The DVE carry-lane budget allows only 6
    distinct leaves, so the clamp is built without the Zero constant:
    v = min(t + |t|, 2) = 2*clamp(t, 0, 1); host pre-scales coefficients
    by 1/2^d to compensate."""
    from concourse import dve_ops
    from concourse.dve_spec import (
        Spec, Src0, Src1, C0, C1, C2, One, minn, sq, lower, Bin, AluOp,
        _has_src1 as has_src1,
    )
    from concourse.dve_uop import DveOpSpec

    if any(op.name == "BSPL_AB_ANT" for op in dve_ops.OPS):
        ab = next(op for op in dve_ops.OPS if op.name == "BSPL_AB_ANT")
        cc = next(op for op in dve_ops.OPS if op.name == "BSPL_C_ANT")
        return ab, cc

    t1 = Src0 - C2
    v1 = minn(t1 + Bin(AluOp.ABSOLUTE_VALUE, t1, t1), One + One)
    body_ab = Src1 + v1 * (C0 + C1 * v1)        # acc + a'*v + b'*v^2
    t2 = Src0 - C2
    v2 = minn(t2 + Bin(AluOp.ABSOLUTE_VALUE, t2, t2), One + One)
    body_c = Src1 + (sq(v2) * v2) * C0          # acc + c'*v^3

    def _vv(in0, imm2):
        tt = in0.astype(np.float32) - np.float32(imm2)
        return np.minimum(tt + np.abs(tt), np.float32(2.0)).astype(np.float32)

    def ref_ab(in0, in1, s0, s1, imm2):
        vv = _vv(in0, imm2)
        return (in1 + vv * (s0 + s1 * vv)).astype(np.float32)

    def ref_c(in0, in1, s0, s1, imm2):
        vv = _vv(in0, imm2)
        return (in1 + (vv * vv * vv) * s0).astype(np.float32)

    def _mk(name, spec):
        # compute the pinned table hashes for this repo's lowerer
        shas = {}
        for ver in ("v3", "v4"):
            probe = DveOpSpec(name=name, opcode=0,
                              uops=lower(spec, ver=ver), rd1_en=has_src1(spec))
            shas[ver] = probe.sha(ver)
        op = dve_ops.DveOp(name, spec, subdim=False, uops_sha=shas)
        dve_ops.OPS.append(op)
        dve_ops.CUSTOM_DVE_SPECS[name] = spec
        row = dve_ops._CUSTOM_DVE_ROW_BASE + len(dve_ops.OPS) - 1
        assert row < 0x20
        dve_ops._SUB_OPCODE_FOR_NAME[name] = row
        return op

    ab = _mk("BSPL_AB_ANT", Spec(body=body_ab, reference=ref_ab))
    cc = _mk("BSPL_C_ANT", Spec(body=body_c, reference=ref_c))
    return ab, cc


# ---------------------------------------------------------------------------
# Bass kernel
# ---------------------------------------------------------------------------

_CACHE = {}


def _build_module(body_reps=1, nj=NJ):
    key = ("nc", body_reps, nj)
    if key in _CACHE:
        return _CACHE[key]
    op_ab, op_c = _register_ops()
    from concourse import masks

    nc = bacc.Bacc("TRN2", target_bir_lowering=False, debug=False,
                   num_devices=NCORES)
    x_in = nc.dram_tensor("x", [BSH, FSH], F32, kind="ExternalInput").ap()
    cpb_in = nc.dram_tensor("cpb", [NK + 1, FSH], F32, kind="ExternalInput").ap()
    w2_in = nc.dram_tensor("w2", [NK + 1, 3 * NJ + 1], F32,
                           kind="ExternalInput").ap()
    y_out = nc.dram_tensor("y", [BSH, FSH], F32, kind="ExternalOutput").ap()

    NCHUNK = BSH // 128   # 8 transpose chunks

    import contextlib
    with tile.TileContext(nc) as tc:
        with contextlib.ExitStack() as _st:
            const_pool = _st.enter_context(tc.tile_pool(name="const", bufs=1))
            xin_pool = _st.enter_context(tc.tile_pool(name="xin", bufs=4))
            big_pool = _st.enter_context(tc.tile_pool(name="big", bufs=1))
            psum_pool = _st.enter_context(
                tc.tile_pool(name="ps", bufs=2, space="PSUM"))
            psum_out_pool = _st.enter_context(
                tc.tile_pool(name="pso", bufs=2, space="PSUM"))
            psum_g_pool = _st.enter_context(
                tc.tile_pool(name="psg", bufs=1, space="PSUM"))
            if body_reps > 1:
                _st.enter_context(tc.For_i(0, body_reps, 1))
            # --- coefficient table: gtab[f, col] = (cpb.T @ w2)[f, col] ---
            cpb_sb = const_pool.tile([NK + 1, FSH], F32)
            nc.sync.dma_start(cpb_sb[:], cpb_in[:])
            w2_sb = const_pool.tile([NK + 1, 3 * NJ + 1], F32)
            nc.sync.dma_start(w2_sb[:], w2_in[:])
            g_ps = psum_g_pool.tile([FSH, 3 * NJ + 1], F32)
            nc.tensor.matmul(g_ps[:], cpb_sb[:], w2_sb[:])
            gtab = const_pool.tile([FSH, 3 * NJ + 1], F32)
            nc.scalar.copy(gtab[:], g_ps[:])

            ident = const_pool.tile([128, 128], F32)
            masks.make_identity(nc, ident[:])

            # --- load x, transpose to (feature, batch), scale to s = 55 x ---
            s_t = big_pool.tile([FSH, BSH], F32)
            for i in range(NCHUNK):
                xt = xin_pool.tile([128, FSH], F32)
                nc.sync.dma_start(xt[:], x_in[bass.ts(i, 128), :])
                pt = psum_pool.tile([FSH, 128], F32)
                nc.tensor.transpose(pt[:], xt[:], ident[:])
                if i % 2 == 0:
                    nc.scalar.mul(s_t[:, bass.ts(i, 128)], pt[:], 55.0)
                else:
                    nc.vector.tensor_scalar_mul(s_t[:, bass.ts(i, 128)],
                                                pt[:], 55.0)

            # --- telescoped chains (NACC independent chains hide op latency) ---
            NACC = int(__import__("os").environ.get("BSPL_NACC", "8"))
            accs_ab = [big_pool.tile([FSH, BSH], F32, name=f"accab{i}",
                                     tag=f"accab{i}") for i in range(NACC // 2)]
            accs_c = [big_pool.tile([FSH, BSH], F32, name=f"accc{i}",
                                    tag=f"accc{i}") for i in range(NACC // 2)]
            nc.scalar.activation(accs_ab[0][:], s_t[:],
                                 mybir.ActivationFunctionType.Identity,
                                 bias=gtab[:, 165:166], scale=0.0)
            for t in accs_ab[1:] + accs_c:
                nc.gpsimd.memset(t[:], 0.0)
            nh = NACC // 2
            for jp in range(nj):
                t_ab = accs_ab[jp % nh]
                t_c = accs_c[jp % nh]
                nc.vector._custom_dve(
                    op_ab, out=t_ab[:], in0=s_t[:], in1=t_ab[:],
                    s0=gtab[:, jp:jp + 1], s1=gtab[:, 55 + jp:56 + jp],
                    imm2=float(jp))
                nc.vector._custom_dve(
                    op_c, out=t_c[:], in0=s_t[:], in1=t_c[:],
                    s0=gtab[:, 110 + jp:111 + jp], imm2=float(jp))
            # merge the chain accumulators; split adds DVE/GPSIMD (DVE is the
            # critical path, GPSIMD absorbs ~1/3 at its slower 2-input rate)
            allacc = accs_ab + accs_c
            k = 0
            while len(allacc) > 1:
                nxt = []
                for i in range(0, len(allacc) - 1, 2):
                    eng = nc.gpsimd if (k % 3 == 2) else nc.vector
                    eng.tensor_add(allacc[i][:], allacc[i][:],
                                   allacc[i + 1][:])
                    k += 1
                    nxt.append(allacc[i])
                if len(allacc) % 2:
                    nxt.append(allacc[-1])
                allacc = nxt
            acc_ab = allacc[0]

            # --- transpose back and store ---
            for i in range(NCHUNK):
                po = psum_out_pool.tile([128, FSH], F32)
                nc.tensor.transpose(po[:], acc_ab[:, bass.ts(i, 128)], ident[:])
                yo = xin_pool.tile([128, FSH], F32, tag="yout")
                # DVE is idle after the chains: split PSUM->SBUF copies ACT/DVE
                if i % 2 == 0:
                    nc.scalar.copy(yo[:], po[:])
                else:
                    nc.vector.tensor_copy(yo[:], po[:])
                nc.sync.dma_start(y_out[bass.ts(i, 128), :], yo[:])

    nc.compile()
    _CACHE[key] = nc
    return nc


# ---------------------------------------------------------------------------
# Public entry point
# ---------------------------------------------------------------------------

def _make_in_maps(x, control_p, bias):
    x = np.ascontiguousarray(x, dtype=np.float32)
    control_p = np.ascontiguousarray(control_p, dtype=np.float32)
    bias = np.ascontiguousarray(bias, dtype=np.float32)
    assert x.shape == (BATCH, NF) and control_p.shape == (NK, NF)
    w2 = _make_w2()
    in_maps, slots = [], []
    for c in range(NCORES):
        fh, bq = c // 4, c % 4
        fsl = slice(fh * FSH, (fh + 1) * FSH)
        bsl = slice(bq * BSH, (bq + 1) * BSH)
        cpb = np.concatenate([control_p[:, fsl], bias[None, fsl]], axis=0)
        in_maps.append({
            "x": np.ascontiguousarray(x[bsl, fsl]),
            "cpb": np.ascontiguousarray(cpb),
            "w2": w2,
        })
        slots.append((bsl, fsl))
    return in_maps, slots


def kernel(x, control_p, bias):
    nc = _build_module()
    in_maps, slots = _make_in_maps(x, control_p, bias)
    res = run_bass_kernel_spmd(nc, in_maps, list(range(NCORES)))

    out = np.empty((BATCH, NF), dtype=np.float32)
    for c, (bsl, fsl) in enumerate(slots):
        out[bsl, fsl] = res.results[c]["y"]
    return out
